# revision 33
# baseline (speedup 1.0000x reference)
"""AECF multimodal fusion kernel for 8 TRN2 NeuronCores.

Strategy:
  - Host-side routing (part of sharding): rows are sorted into three branch
    groups (both modalities present / only-image / only-text) using the same
    norm>1e-6 predicate as the reference. Each group is dealt evenly across
    the 8 cores and padded to a tile multiple; the NEFF is compiled with the
    actual per-core group sizes (compile happens inside kernel(), after the
    inputs are known), so the graph is static and identical on all cores.
  - Tile-major feature-major layout: inputs are shipped as flat [128, 4*n]
    bf16 tile blocks (contiguous per partition per tile; img/txt halves of a
    both-tile DMA'd from the sync/scalar queues), outputs leave as flat
    [80, n] f32 tile blocks.  The last tile of each group is short (groups
    pad to a multiple of 16, not T), and startup DMA priority is
    wie/wte + tile 0 before the remaining weights and deeper prefetch.
  - bf16 storage/compute, f32 PSUM accumulation.
  - Algebra: scores only enter through a 2-way softmax, so
    attn_img = sigmoid(s_img - s_txt) with s = enc @ Wk_eff,
    Wk_eff[:, h] = Wk[:, 64h:64h+64] @ q[h] / 8 (bk cancels in the diff).
    There is no nonlinearity between the post-attention projection and the
    classifier's first layer, so Wo, W_fp, Wc1 merge into Wofc1 [256,256]:
      h1pre = pooled @ Wofc1 + bh1
            = enc_t @ (Wv@Wofc1) + (A * v_d) @ Wofc1 + bh1
    with v_d = (enc_i - enc_t) @ Wv and A = head-broadcast(attn) (PE matmul
    with a 0/1 expander).  v_txt is never materialized.  Only-branches use
    Wipc1 = W_ip@Wc1 / Wtpc1 = W_tp@Wc1.
  - Depth-3 software pipeline per both-tile, ordered so the PE queue (the
    bottleneck engine) never waits on ACT/DVE round trips; PSUM tiles span
    2 banks so evacuations/relus are single ops over [128, 1024].
"""

import os
import sys

if "/opt/trn_rl_repo" not in sys.path:
    sys.path.insert(0, "/opt/trn_rl_repo")

import numpy as np
import ml_dtypes

import concourse.bass as bass
import concourse.bacc as bacc
import concourse.tile as tile
from concourse import mybir
from concourse.bass_utils import run_bass_kernel_spmd

BF = mybir.dt.bfloat16
F32 = mybir.dt.float32
AF = mybir.ActivationFunctionType
OP = mybir.AluOpType

H = 256
ID = 512
TD = 512
NCLS = 80
NH = 4
HD = 64
B = 131072
NCORES = 8
T = 512  # batch-tile (free-dim) size; one psum bank of f32 per 128-chunk

LAST_EXEC_NS = None
LAST_PROFILE = None

_GRAPH_CACHE = {}


def _ntl(n):
    """Number of tiles and last-tile length for a group of n columns."""
    ntiles = (n + T - 1) // T
    tl_last = n - (ntiles - 1) * T if ntiles else 0
    return ntiles, tl_last


def _build_graph(nb, ni, nt, zero_bias):
    """Build the SPMD graph for per-core group column counts nb/ni/nt
    (each a multiple of 16, possibly 0). Inputs/outputs are flat
    tile-major blocks so the final tile of each group can be short."""
    nc = bacc.Bacc()
    nbt, tlb = _ntl(nb)
    nit, tli = _ntl(ni)
    ntt, tlt = _ntl(nt)

    # ---- DRAM I/O ----
    dram = {}
    if nbt:
        dram["xb"] = nc.dram_tensor("xb", [128, 8 * nb], BF, kind="ExternalInput")
        dram["outb"] = nc.dram_tensor("outb", [NCLS, nb], F32, kind="ExternalOutput")
    if nit:
        dram["xi_img"] = nc.dram_tensor("xi_img", [128, 4 * ni], BF, kind="ExternalInput")
        dram["outi"] = nc.dram_tensor("outi", [NCLS, ni], F32, kind="ExternalOutput")
    if ntt:
        dram["xt_txt"] = nc.dram_tensor("xt_txt", [128, 4 * nt], BF, kind="ExternalInput")
        dram["outt"] = nc.dram_tensor("outt", [NCLS, nt], F32, kind="ExternalOutput")

    wspec = {
        "wie": ([128, 4, H], BF),
        "wte": ([128, 4, H], BF),
        "wkeff": ([128, 2, NH], BF),
        "emat": ([NH, 2, 128], BF),
        "wv": ([128, 2, H], BF),
        "wvofc1": ([128, 2, H], BF),
        "wofc1": ([128, 2, H], BF),
        "wipc1": ([128, 2, H], BF),
        "wtpc1": ([128, 2, H], BF),
        "wc2": ([128, 2, NCLS], BF),
        "bie": ([128, 2], F32),
        "bte": ([128, 2], F32),
        "bh1b": ([128, 2], F32),
        "bh1i": ([128, 2], F32),
        "bh1t": ([128, 2], F32),
        "bc2": ([128, 1], F32),
    }
    for name, (shape, dt) in wspec.items():
        dram[name] = nc.dram_tensor(name, shape, dt, kind="ExternalInput")

    with tile.TileContext(nc) as tc:
        with (
            tc.tile_pool(name="wpool", bufs=1) as wpool,
            tc.tile_pool(name="work", bufs=2) as wp,
            tc.tile_pool(name="psum", bufs=1, space="PSUM") as pp,
        ):
            # DMA priority at startup: the first encoder tile needs only
            # wie/wte + tile 0's inputs, so those ride the sync/scalar
            # queues first; the remaining weights (gpsimd) and deeper
            # prefetch are emitted from the pipeline prologue below.
            w = {}

            def load_w(names, eng):
                for name in names:
                    shape, dt = wspec[name]
                    w[name] = wpool.tile(shape, dt, tag=name, name=name)
                    eng.dma_start(w[name][:], dram[name][:])

            load_w(["wie"], nc.sync)
            load_w(["wte"], nc.scalar)
            w_rest = [n for n in wspec if n not in ("wie", "wte")]
            _wstate = {"loaded": False}

            def preload():
                """Emit the bulk weight DMAs once, after the first input
                tile's DMAs so tile 0 wins the ring arbitration."""
                if not _wstate["loaded"]:
                    _wstate["loaded"] = True
                    load_w(w_rest, nc.gpsimd)

            def relu_evac(dst, ps, tag2, tl):
                """psum [128,2,:tl] -> sbuf bf16 with relu (+bias per m-half)."""
                if zero_bias:
                    nc.scalar.activation(dst[:, :, :tl], ps[:, :, :tl], AF.Relu)
                else:
                    for m in range(2):
                        nc.scalar.activation(
                            dst[:, m, :tl], ps[:, m, :tl], AF.Relu,
                            bias=w[tag2][:, m : m + 1],
                        )

            # ================= both-modality pipeline =================
            # Stages per tile i:
            #   S1: input DMAs      S2: encoders (PE 16) + relus + denc
            #   S3a: wkeff (PE 2) + sigmoid     S3b: vd (PE 4) + emat (PE 2)
            #        + vd evac + tmp = A*vd
            #   S4: h1pre (PE 8: enc_t@Wvofc1 + tmp@Wofc1) + h1 relu
            #   S5: wc2 (PE 2) + out evac + out DMA
            # Iteration i emits: S3a(i), S2(i+1), S3b(i), S4(i-1), S5(i-2),
            # S1(i+2) so the in-order PE queue never waits on ACT/DVE.
            X, ENC, DNC, ATT, TMP, H1 = {}, {}, {}, {}, {}, {}

            def b_tl(i):
                return T if i < nbt - 1 else tlb

            def b_s1(i):
                tl = b_tl(i)
                off = 8 * T * i
                xb = wp.tile([128, 8, T], BF, tag="xi", bufs=5, name="xb")
                nc.sync.dma_start(
                    xb[:, 0:4, :tl],
                    dram["xb"][:, off : off + 4 * tl].rearrange(
                        "p (k c) -> p k c", k=4),
                )
                nc.scalar.dma_start(
                    xb[:, 4:8, :tl],
                    dram["xb"][:, off + 4 * tl : off + 8 * tl].rearrange(
                        "p (k c) -> p k c", k=4),
                )
                X[i] = xb

            def b_s2(i):
                tl = b_tl(i)
                xb = X.pop(i)
                pei = pp.tile([128, 2, T], F32, tag="ps_big", bufs=3, name="pei")
                for m in range(2):
                    for k in range(4):
                        nc.tensor.matmul(
                            pei[:, m, :tl], w["wie"][:, k, m * 128 : (m + 1) * 128],
                            xb[:, k, :tl], start=(k == 0), stop=(k == 3),
                        )
                enci = wp.tile([128, 2, T], BF, tag="enci", bufs=2, name="enci")
                relu_evac(enci, pei, "bie", tl)
                pet = pp.tile([128, 2, T], F32, tag="ps_big", bufs=3, name="pet")
                for m in range(2):
                    for k in range(4):
                        nc.tensor.matmul(
                            pet[:, m, :tl], w["wte"][:, k, m * 128 : (m + 1) * 128],
                            xb[:, 4 + k, :tl], start=(k == 0), stop=(k == 3),
                        )
                enct = wp.tile([128, 2, T], BF, tag="enct", bufs=3, name="enct")
                relu_evac(enct, pet, "bte", tl)
                denc = wp.tile([128, 2, T], BF, tag="denc", bufs=3, name="denc")
                nc.vector.tensor_tensor(denc[:, :, :tl], enci[:, :, :tl],
                                        enct[:, :, :tl], op=OP.subtract)
                ENC[i] = enct
                DNC[i] = denc

            def b_s3a(i):
                tl = b_tl(i)
                d = DNC[i]
                pss = pp.tile([128, T], F32, tag="ps_small", bufs=2, name="pss")
                for k in range(2):
                    nc.tensor.matmul(pss[:NH, :tl], w["wkeff"][:, k, :],
                                     d[:, k, :tl], start=(k == 0), stop=(k == 1))
                att = wp.tile([NH, T], BF, tag="att", bufs=3, name="att")
                nc.scalar.activation(att[:, :tl], pss[:NH, :tl], AF.Sigmoid)
                ATT[i] = att

            def b_s3b(i):
                tl = b_tl(i)
                d = DNC.pop(i)
                att = ATT.pop(i)
                psv = pp.tile([128, 2, T], F32, tag="ps_big", bufs=3, name="psv")
                for m in range(2):
                    for k in range(2):
                        nc.tensor.matmul(
                            psv[:, m, :tl], w["wv"][:, k, m * 128 : (m + 1) * 128],
                            d[:, k, :tl], start=(k == 0), stop=(k == 1),
                        )
                psa = pp.tile([128, 2, T], F32, tag="ps_big", bufs=3, name="psa")
                for m in range(2):
                    nc.tensor.matmul(psa[:, m, :tl], w["emat"][:, m, :],
                                     att[:, :tl], start=True, stop=True)
                vd = wp.tile([128, 2, T], BF, tag="vd", bufs=2, name="vd")
                nc.vector.tensor_copy(vd[:, :, :tl], psv[:, :, :tl])
                tmp = wp.tile([128, 2, T], BF, tag="tmp", bufs=3, name="tmp")
                nc.vector.tensor_tensor(tmp[:, :, :tl], psa[:, :, :tl],
                                        vd[:, :, :tl], op=OP.mult)
                TMP[i] = tmp

            def b_s4(i):
                tl = b_tl(i)
                enct = ENC.pop(i)
                tmp = TMP.pop(i)
                psh = pp.tile([128, 2, T], F32, tag="ps_big", bufs=3, name="psh")
                for m in range(2):
                    ms = slice(m * 128, (m + 1) * 128)
                    nc.tensor.matmul(psh[:, m, :tl], w["wvofc1"][:, 0, ms],
                                     enct[:, 0, :tl], start=True, stop=False)
                    nc.tensor.matmul(psh[:, m, :tl], w["wvofc1"][:, 1, ms],
                                     enct[:, 1, :tl], start=False, stop=False)
                    nc.tensor.matmul(psh[:, m, :tl], w["wofc1"][:, 0, ms],
                                     tmp[:, 0, :tl], start=False, stop=False)
                    nc.tensor.matmul(psh[:, m, :tl], w["wofc1"][:, 1, ms],
                                     tmp[:, 1, :tl], start=False, stop=True)
                h1 = wp.tile([128, 2, T], BF, tag="h1", bufs=3, name="h1")
                relu_evac(h1, psh, "bh1b", tl)
                H1[i] = h1

            def b_s5(i):
                tl = b_tl(i)
                h1 = H1.pop(i)
                pso = pp.tile([128, T], F32, tag="ps_small", bufs=2, name="pso")
                for k in range(2):
                    nc.tensor.matmul(pso[:NCLS, :tl], w["wc2"][:, k, :],
                                     h1[:, k, :tl], start=(k == 0), stop=(k == 1))
                osb = wp.tile([NCLS, T], F32, tag="osb", bufs=3, name="osb")
                if zero_bias:
                    nc.vector.tensor_copy(osb[:, :tl], pso[:NCLS, :tl])
                else:
                    nc.vector.tensor_scalar_add(osb[:, :tl], pso[:NCLS, :tl],
                                                w["bc2"][:NCLS, :])
                nc.gpsimd.dma_start(dram["outb"][:, T * i : T * i + tl],
                                    osb[:, :tl])

            def both_pipe():
                b_s1(0)
                if nbt > 1:
                    b_s1(1)
                preload()
                for i0 in range(2, min(4, nbt)):
                    b_s1(i0)
                b_s2(0)
                if nbt > 1:
                    b_s2(1)
                for i in range(nbt + 2):
                    if i < nbt:
                        b_s3a(i)
                    if 0 < i and i + 1 < nbt:
                        b_s2(i + 1)
                    if i < nbt:
                        b_s3b(i)
                    if 0 <= i - 1 < nbt:
                        b_s4(i - 1)
                    if 0 <= i - 2 < nbt:
                        b_s5(i - 2)
                    if i + 4 < nbt:
                        b_s1(i + 4)

            # ================= single-modality pipelines =================
            # out = relu(enc @ Wpc1 + bh1) @ Wc2 + bc2, enc = relu(x@We+be)
            def only_pipe(x_dram, out_dram, ntiles, tl_last, wenc, benc, wproj,
                          bproj, out_eng):
                XO, ENO, HO = {}, {}, {}

                def o_tl(j):
                    return T if j < ntiles - 1 else tl_last

                def o_s1(j):
                    tl = o_tl(j)
                    off = 4 * T * j
                    xo = wp.tile([128, 4, T], BF, tag="xi", bufs=5, name="xo")
                    nc.sync.dma_start(
                        xo[:, :, :tl],
                        x_dram[:, off : off + 4 * tl].rearrange(
                            "p (k c) -> p k c", k=4),
                    )
                    XO[j] = xo

                def o_s2(j):
                    tl = o_tl(j)
                    xo = XO.pop(j)
                    pe = pp.tile([128, 2, T], F32, tag="ps_big", bufs=3, name="peo")
                    for m in range(2):
                        for k in range(4):
                            nc.tensor.matmul(
                                pe[:, m, :tl], w[wenc][:, k, m * 128 : (m + 1) * 128],
                                xo[:, k, :tl], start=(k == 0), stop=(k == 3),
                            )
                    eno = wp.tile([128, 2, T], BF, tag="enct", bufs=3, name="eno")
                    relu_evac(eno, pe, benc, tl)
                    ENO[j] = eno

                def o_s3(j):
                    tl = o_tl(j)
                    eno = ENO.pop(j)
                    psh = pp.tile([128, 2, T], F32, tag="ps_big", bufs=3, name="psho")
                    for m in range(2):
                        ms = slice(m * 128, (m + 1) * 128)
                        for k in range(2):
                            nc.tensor.matmul(psh[:, m, :tl], w[wproj][:, k, ms],
                                             eno[:, k, :tl], start=(k == 0),
                                             stop=(k == 1))
                    h1 = wp.tile([128, 2, T], BF, tag="h1", bufs=3, name="h1o")
                    if zero_bias:
                        nc.vector.tensor_scalar_max(h1[:, :, :tl], psh[:, :, :tl],
                                                    0.0)
                    else:
                        for m in range(2):
                            nc.scalar.activation(
                                h1[:, m, :tl], psh[:, m, :tl], AF.Relu,
                                bias=w[bproj][:, m : m + 1],
                            )
                    HO[j] = h1

                def o_s4(j):
                    tl = o_tl(j)
                    h1 = HO.pop(j)
                    pso = pp.tile([128, T], F32, tag="ps_small", bufs=2, name="psoo")
                    for k in range(2):
                        nc.tensor.matmul(pso[:NCLS, :tl], w["wc2"][:, k, :],
                                         h1[:, k, :tl], start=(k == 0),
                                         stop=(k == 1))
                    osb = wp.tile([NCLS, T], F32, tag="osb", bufs=3, name="osbo")
                    if zero_bias:
                        nc.vector.tensor_copy(osb[:, :tl], pso[:NCLS, :tl])
                    else:
                        nc.vector.tensor_scalar_add(osb[:, :tl], pso[:NCLS, :tl],
                                                    w["bc2"][:NCLS, :])
                    out_eng.dma_start(out_dram[:, T * j : T * j + tl], osb[:, :tl])

                o_s1(0)
                preload()
                for j0 in range(1, min(3, ntiles)):
                    o_s1(j0)
                o_s2(0)
                for j in range(ntiles + 1):
                    if j + 1 < ntiles:
                        o_s2(j + 1)
                    if j < ntiles:
                        o_s3(j)
                    if 0 <= j - 1 < ntiles:
                        o_s4(j - 1)
                    if j + 3 < ntiles:
                        o_s1(j + 3)

            # only-img first: it needs just wie + one 512KB tile to start,
            # and its compute window lets the DMA system get ahead on the
            # both-pipeline's 1MB tiles.
            if nit:
                only_pipe(dram["xi_img"], dram["outi"], nit, tli, "wie", "bie",
                          "wipc1", "bh1i", nc.gpsimd)
            if nbt:
                both_pipe()
            if ntt:
                only_pipe(dram["xt_txt"], dram["outt"], ntt, tlt, "wte", "bte",
                          "wtpc1", "bh1t", nc.scalar)
            preload()

    nc.compile()
    return nc


def _prep_weights(inp):
    """Host-side weight prep: fold/merge/transpose into the device layouts."""
    f32 = np.float32
    q = (inp["fusion_query"].reshape(1, H).astype(f32) @ inp["Wq"] + inp["bq"]).reshape(
        NH, HD
    )
    wkeff = np.zeros((H, NH), f32)
    for h in range(NH):
        wkeff[:, h] = inp["Wk"][:, h * HD : (h + 1) * HD] @ q[h] / np.sqrt(HD)
    wof = inp["Wo"].astype(f32) @ inp["W_fp"]
    bof = inp["bo"].astype(f32) @ inp["W_fp"] + inp["b_fp"]
    wofc1 = wof @ inp["Wc1"]
    wvofc1 = inp["Wv"].astype(f32) @ wofc1
    bh1b = inp["bv"].astype(f32) @ wofc1 + bof @ inp["Wc1"] + inp["bc1"]
    wipc1 = inp["W_ip"].astype(f32) @ inp["Wc1"]
    bh1i = inp["b_ip"].astype(f32) @ inp["Wc1"] + inp["bc1"]
    wtpc1 = inp["W_tp"].astype(f32) @ inp["Wc1"]
    bh1t = inp["b_tp"].astype(f32) @ inp["Wc1"] + inp["bc1"]
    emat = np.zeros((NH, H), f32)
    for h in range(NH):
        emat[h, h * HD : (h + 1) * HD] = 1.0

    def ktile(a, kt):  # [K, M] -> [128, kt, M]
        return np.ascontiguousarray(
            a.reshape(kt, 128, a.shape[1]).transpose(1, 0, 2)
        )

    bf = ml_dtypes.bfloat16
    out = {
        "wie": ktile(inp["W_ie"], 4).astype(bf),
        "wte": ktile(inp["W_te"], 4).astype(bf),
        "wkeff": ktile(wkeff, 2).astype(bf),
        "emat": np.ascontiguousarray(emat.reshape(NH, 2, 128)).astype(bf),
        "wv": ktile(inp["Wv"].astype(f32), 2).astype(bf),
        "wvofc1": ktile(wvofc1, 2).astype(bf),
        "wofc1": ktile(wofc1, 2).astype(bf),
        "wipc1": ktile(wipc1, 2).astype(bf),
        "wtpc1": ktile(wtpc1, 2).astype(bf),
        "wc2": ktile(inp["Wc2"].astype(f32), 2).astype(bf),
        "bie": np.ascontiguousarray(inp["b_ie"].reshape(2, 128).T).astype(f32),
        "bte": np.ascontiguousarray(inp["b_te"].reshape(2, 128).T).astype(f32),
        "bh1b": np.ascontiguousarray(bh1b.reshape(2, 128).T).astype(f32),
        "bh1i": np.ascontiguousarray(bh1i.reshape(2, 128).T).astype(f32),
        "bh1t": np.ascontiguousarray(bh1t.reshape(2, 128).T).astype(f32),
        "bc2": np.ascontiguousarray(
            np.pad(inp["bc2"].astype(f32), (0, 128 - NCLS)).reshape(128, 1)
        ),
    }
    return out


def _split_pad(idx):
    """Split index array across cores evenly; pad each core's slice to a
    multiple of 16 with -1. Returns list of per-core padded index arrays
    (all the same length)."""
    per = [idx[c::NCORES] for c in range(NCORES)]
    n = max(len(p) for p in per)
    npad = ((n + 15) // 16) * 16 if n else 0
    out = []
    for p in per:
        a = np.full(npad, -1, dtype=np.int64)
        a[: len(p)] = p
        out.append(a)
    return out


def _tile_blocks(x_bf, idx):
    """Rows idx of x (with -1 -> zero row) as a list of feature-major
    tile blocks [128, 4, tl]: block[j][p, k, c] = x[idx[j*T+c], k*128+p]."""
    n = len(idx)
    g = np.zeros((n, x_bf.shape[1]), dtype=x_bf.dtype)
    valid = idx >= 0
    g[valid] = x_bf[idx[valid]]
    ntiles, tl_last = _ntl(n)
    blocks = []
    for j in range(ntiles):
        tl = T if j < ntiles - 1 else tl_last
        blk = g[j * T : j * T + tl]
        blocks.append(np.ascontiguousarray(blk.reshape(tl, 4, 128).transpose(2, 1, 0)))
    return blocks


def _flat_single(x_bf, idx):
    """[128, 4*n] flat tile-major input blocks."""
    return np.concatenate(
        [b.reshape(128, -1) for b in _tile_blocks(x_bf, idx)], axis=1
    )


def _flat_pair(img_bf, txt_bf, idx):
    """[128, 8*n]: per tile, img block cols then txt block cols."""
    bi = _tile_blocks(img_bf, idx)
    bt = _tile_blocks(txt_bf, idx)
    return np.concatenate(
        [np.concatenate([a.reshape(128, -1), b.reshape(128, -1)], axis=1)
         for a, b in zip(bi, bt)],
        axis=1,
    )


def _ntff_hook():
    """Build the (output_dir, device_ids) -> contextmanager NTFF profile
    hook directly via ctypes on the axon PJRT .so (the image's antenv lacks
    axon_hooks, so the boot-time registration was skipped)."""
    import ctypes
    import contextlib

    so_path = "/opt/axon/libaxon_pjrt.so"
    lib = ctypes.CDLL(so_path)
    if not hasattr(lib, "axon_start_nrt_profile"):
        return None
    lib.axon_start_nrt_profile.argtypes = [
        ctypes.POINTER(ctypes.c_int64),
        ctypes.c_size_t,
    ]
    lib.axon_start_nrt_profile.restype = ctypes.c_int64
    lib.axon_stop_nrt_profile.argtypes = [ctypes.c_char_p]
    lib.axon_stop_nrt_profile.restype = ctypes.c_int64

    @contextlib.contextmanager
    def _hook(output_dir, device_ids):
        import jax

        jax.devices()
        if device_ids:
            ids = (ctypes.c_int64 * len(device_ids))(*device_ids)
            rc = lib.axon_start_nrt_profile(ids, len(device_ids))
        else:
            rc = lib.axon_start_nrt_profile(None, 0)
        if rc != 0:
            raise RuntimeError(f"axon_start_nrt_profile rc={rc}")
        try:
            yield
        finally:
            n = lib.axon_stop_nrt_profile(str(output_dir).encode())
            print(f"profile: {n} file(s) written to {output_dir}", file=sys.stderr)

    return _hook


def _profiled_run(nc, in_maps):
    """Run via PJRT with NTFF profiling; parse exec_time_ns from the trace."""
    import tempfile
    import glob as _glob

    from concourse import bass2jax
    from concourse._compat import FishPath
    import gauge.profiler

    hook = _ntff_hook()
    tmpdir = tempfile.mkdtemp(prefix="aecf_prof_")
    if hook is None:
        results = bass2jax.run_bass_via_pjrt(nc, in_maps, n_cores=NCORES)
        return results, None, None
    with hook(tmpdir, [0]):
        results = bass2jax.run_bass_via_pjrt(nc, in_maps, n_cores=NCORES)
    ntffs = _glob.glob(os.path.join(tmpdir, "*_body*.ntff"))
    if not ntffs:
        print(f"no NTFFs in {tmpdir}: {sorted(os.listdir(tmpdir))}", file=sys.stderr)
        return results, None, None
    prof = gauge.profiler.Profile(
        profile_path=FishPath(tmpdir),
        kernel_dev_mode=True,
        profile_on_exit=False,
        bass_kernel=nc.m,
        offline_processing=True,
        fname="*_body*",
        metadata={},
    )
    try:
        pres = prof.to_perfetto(model_index=(0,))
        exec_ns = pres[0].exec_time_ns if pres else None
        pjson = prof.json_path(0).path if pres else None
    except Exception as e:
        print(f"profile parse failed: {e}", file=sys.stderr)
        return results, None, None
    return results, exec_ns, pjson


def kernel(**inputs):
    global LAST_EXEC_NS, LAST_PROFILE
    img = np.asarray(inputs["image_features"], dtype=np.float32)
    txt = np.asarray(inputs["text_features"], dtype=np.float32)

    pres_i = np.linalg.norm(img, axis=1) > 1e-6
    pres_t = np.linalg.norm(txt, axis=1) > 1e-6
    both = pres_i & pres_t
    oi = pres_i & ~pres_t
    ot = ~pres_i & pres_t
    none = ~pres_i & ~pres_t

    idx_b = _split_pad(np.nonzero(both)[0])
    idx_i = _split_pad(np.nonzero(oi)[0])
    idx_t = _split_pad(np.nonzero(ot)[0])
    nb, ni, nt = len(idx_b[0]), len(idx_i[0]), len(idx_t[0])

    bias_names = ("b_ie", "b_te", "bv", "bo", "b_fp", "b_ip", "b_tp", "bc1", "bc2")
    zero_bias = all(not np.any(np.asarray(inputs[n])) for n in bias_names)
    key = (nb, ni, nt, zero_bias)
    if key not in _GRAPH_CACHE:
        _GRAPH_CACHE[key] = _build_graph(nb, ni, nt, zero_bias)
    nc = _GRAPH_CACHE[key]

    wmap = _prep_weights(inputs)
    bf = ml_dtypes.bfloat16
    img_bf = img.astype(bf)
    txt_bf = txt.astype(bf)

    in_maps = []
    for c in range(NCORES):
        m = dict(wmap)
        if nb:
            m["xb"] = _flat_pair(img_bf, txt_bf, idx_b[c])
        if ni:
            m["xi_img"] = _flat_single(img_bf, idx_i[c])
        if nt:
            m["xt_txt"] = _flat_single(txt_bf, idx_t[c])
        in_maps.append(m)

    trace = bool(int(os.environ.get("KERNEL_PROFILE", "0")))
    if trace:
        results, exec_ns, prof_json = _profiled_run(nc, in_maps)
        LAST_EXEC_NS = exec_ns
        LAST_PROFILE = prof_json

        class _R:
            pass

        res = _R()
        res.results = results
    else:
        res = run_bass_kernel_spmd(nc, in_maps, core_ids=list(range(NCORES)))
        LAST_EXEC_NS = None
        LAST_PROFILE = None

    logits = np.empty((img.shape[0], NCLS), dtype=np.float32)
    for c in range(NCORES):
        r = res.results[c]
        for name, idx in (("outb", idx_b[c]), ("outi", idx_i[c]), ("outt", idx_t[c])):
            if name in r:
                valid = idx >= 0
                logits[idx[valid]] = r[name].T[valid]

    if none.any():
        # reference: fused = 0 -> logits = relu(bc1) @ Wc2 + bc2 (constant)
        row = (
            np.maximum(inputs["bc1"].astype(np.float32), 0.0) @ inputs["Wc2"]
            + inputs["bc2"]
        )
        logits[none] = row
    return logits
